# revision 2
# baseline (speedup 1.0000x reference)
"""Trainium2 Bass kernel: CNModel GNN message passing + common-neighbor scores.

Computes, for N=4096 nodes / E=131072 edges:
    agg  = segment_sum(x[src], dst)          # scatter-add == A @ x (A dense adjacency)
    h    = relu(agg @ W)
    pred = sigmoid(h.T @ h)

Distribution over 8 NeuronCores (all-static SPMD, one NEFF, one launch):
  - host densifies the edge list into A_T[src, dst] (edge counts) and hands
    core m the column block A_T[:, m*512:(m+1)*512]
  - core m computes h_m = relu(A_T_blk.T @ x [@ W]) = rows [m*512, (m+1)*512) of h
  - AllGather(h_m) -> full h on every core (4 MB/rank, bf16)
  - core m computes pred[m*512:(m+1)*512, :] = h[:, blk_m].T @ h with the
    column block selected at runtime from partition_id, sigmoid on PSUM
    eviction, writes its 512-row f32 output block
All matmuls run in bf16 with fp32 PSUM accumulation.
"""

import numpy as np
import ml_dtypes

N_NODES = 4096
N_CORES = 8
P = 128  # SBUF partitions / PE array dim
FREE = 512  # moving-operand free dim == one PSUM bank of f32

_CACHE: dict = {}


def _build_nc(n: int, with_w: bool):
    """Build + compile the SPMD Bass program for n nodes (n % 4096 layout)."""
    import concourse.bacc as bacc
    import concourse.bass as bass
    import concourse.mybir as mybir
    import concourse.tile as tile

    dt = mybir.dt
    AFT = mybir.ActivationFunctionType

    blk = n // N_CORES  # rows of h / out per core
    kt_n = n // P  # contraction tiles
    nt_n = n // FREE  # output column chunks
    mt_n = blk // P  # output row tiles per core

    nc = bacc.Bacc(
        "TRN2", target_bir_lowering=False, debug=False, num_devices=N_CORES
    )
    a_t = nc.dram_tensor("a_t", [n, blk], dt.bfloat16, kind="ExternalInput").ap()
    x = nc.dram_tensor("x", [n, n], dt.bfloat16, kind="ExternalInput").ap()
    w = (
        nc.dram_tensor("w", [n, n], dt.bfloat16, kind="ExternalInput").ap()
        if with_w
        else None
    )
    out = nc.dram_tensor("out", [blk, n], dt.float32, kind="ExternalOutput").ap()

    with tile.TileContext(nc) as tc:
        with (
            tc.tile_pool(name="dram", bufs=1, space="DRAM") as dram_pool,
            tc.tile_pool(name="lhsT", bufs=1) as lhsT_pool,
            tc.tile_pool(name="rhs", bufs=2) as rhs_pool,
            tc.tile_pool(name="ps", bufs=8, space="PSUM") as psum_pool,
            tc.tile_pool(name="ev", bufs=4) as ev_pool,
            tc.tile_pool(name="aux", bufs=2) as aux_pool,
        ):
            h_bounce = dram_pool.tile([blk, n], dt.bfloat16, name="h_bounce")
            h_all = dram_pool.tile(
                [n, n], dt.bfloat16, name="h_all", addr_space="Shared"
            )

            def stream_gemm(lhsT_sb, rhs_dram, evict):
                # out[mt*P + p, nt*FREE + f] = sum_k lhsT_sb[k][:, mt] . rhs[k][:, nt]
                for nt in range(nt_n):
                    rhs_t = rhs_pool.tile(
                        [P, kt_n, FREE], dt.bfloat16, name="rhs_t", tag="rhs"
                    )
                    nc.sync.dma_start(
                        rhs_t[:],
                        rhs_dram[:, nt * FREE : (nt + 1) * FREE].rearrange(
                            "(kt p) f -> p kt f", p=P
                        ),
                    )
                    for mt in range(mt_n):
                        ps = psum_pool.tile([P, FREE], dt.float32, name="ps", tag="ps")
                        for kt in range(kt_n):
                            nc.tensor.matmul(
                                ps[:],
                                lhsT_sb[:, kt, mt * P : (mt + 1) * P],
                                rhs_t[:, kt, :],
                                start=(kt == 0),
                                stop=(kt == kt_n - 1),
                            )
                        evict(nt, mt, ps)

            def evict_h(nt, mt, ps):
                hv = ev_pool.tile([P, FREE], dt.bfloat16, name="hv", tag="ev16")
                nc.scalar.activation(hv[:], ps[:], AFT.Relu)
                nc.sync.dma_start(
                    h_bounce[mt * P : (mt + 1) * P, nt * FREE : (nt + 1) * FREE],
                    hv[:],
                )

            if not with_w:
                # h_m = relu(A_T_blk.T @ x): lhsT = a_t, rhs = x
                at_sb = lhsT_pool.tile(
                    [P, kt_n, blk], dt.bfloat16, name="at_sb", tag="lhsT"
                )
                nc.sync.dma_start(
                    at_sb[:], a_t.rearrange("(kt p) m -> p kt m", p=P)
                )
                stream_gemm(at_sb, x, evict_h)
            else:
                # aggT_blk = x.T @ A_T_blk, kept SBUF-resident as phase-2 lhsT
                art_sb = aux_pool.tile(
                    [P, kt_n, blk], dt.bfloat16, name="art_sb", tag="art", bufs=1
                )
                nc.sync.dma_start(
                    art_sb[:], a_t.rearrange("(kt p) m -> p kt m", p=P)
                )
                aggT_sb = lhsT_pool.tile(
                    [P, kt_n, blk], dt.bfloat16, name="aggT_sb", tag="lhsT"
                )
                for mt0 in range(kt_n):
                    xp = aux_pool.tile([P, kt_n, P], dt.bfloat16, name="xp", tag="xp")
                    nc.sync.dma_start(
                        xp[:],
                        x[:, mt0 * P : (mt0 + 1) * P].rearrange(
                            "(kt p) f -> p kt f", p=P
                        ),
                    )
                    ps0 = psum_pool.tile([P, blk], dt.float32, name="ps0", tag="ps")
                    for kt in range(kt_n):
                        nc.tensor.matmul(
                            ps0[:],
                            xp[:, kt, :],
                            art_sb[:, kt, :],
                            start=(kt == 0),
                            stop=(kt == kt_n - 1),
                        )
                    nc.vector.tensor_copy(aggT_sb[:, mt0, :], ps0[:])
                # h_m = relu(aggT_blk.T @ W)
                stream_gemm(aggT_sb, w, evict_h)

            nc.gpsimd.collective_compute(
                "AllGather",
                mybir.AluOpType.bypass,
                replica_groups=[list(range(N_CORES))],
                ins=[h_bounce.opt()],
                outs=[h_all.opt()],
            )

            # pred[blk_m, :] = h[:, blk_m].T @ h; blk_m from partition id
            rank = nc.partition_id()
            l3 = lhsT_pool.tile([P, kt_n, blk], dt.bfloat16, name="l3", tag="lhsT")
            for kt in range(kt_n):
                nc.sync.dma_start(
                    l3[:, kt, :],
                    h_all[kt * P : (kt + 1) * P, bass.ts(rank, blk)],
                )

            def evict_o(nt, mt, ps):
                ov = ev_pool.tile([P, FREE], dt.float32, name="ov", tag="ev32")
                nc.scalar.activation(ov[:], ps[:], AFT.Sigmoid)
                nc.sync.dma_start(
                    out[mt * P : (mt + 1) * P, nt * FREE : (nt + 1) * FREE],
                    ov[:],
                )

            stream_gemm(l3, h_all, evict_o)

    nc.compile()
    return nc


def _get_nc(n: int, with_w: bool):
    key = (n, with_w)
    if key not in _CACHE:
        _CACHE[key] = _build_nc(n, with_w)
    return _CACHE[key]


def _kernel_impl(x, edge_index, W, n):
    from concourse.bass_utils import run_bass_kernel_spmd

    bf16 = ml_dtypes.bfloat16
    x = np.ascontiguousarray(np.asarray(x, dtype=np.float32))
    W = np.asarray(W, dtype=np.float32)
    ei = np.asarray(edge_index)
    src = np.asarray(ei[0], dtype=np.intp)
    dst = np.asarray(ei[1], dtype=np.intp)

    # densify edges: A_T[s, d] = multiplicity of edge s->d
    a_t = np.zeros((n, n), dtype=np.float32)
    np.add.at(a_t, (src, dst), 1.0)
    a_t16 = a_t.astype(bf16)
    x16 = x.astype(bf16)

    w_is_identity = (
        np.count_nonzero(W) == n and bool((np.diagonal(W) == 1.0).all())
    )
    nc = _get_nc(n, not w_is_identity)

    blk = n // N_CORES
    in_maps = []
    for m in range(N_CORES):
        im = {
            "a_t": np.ascontiguousarray(a_t16[:, m * blk : (m + 1) * blk]),
            "x": x16,
        }
        if not w_is_identity:
            im["w"] = W.astype(bf16)
        in_maps.append(im)

    res = run_bass_kernel_spmd(nc, in_maps, list(range(N_CORES)))
    global LAST_RESULT
    LAST_RESULT = res
    return np.concatenate(
        [np.asarray(res.results[m]["out"]) for m in range(N_CORES)], axis=0
    )


LAST_RESULT = None


def kernel(x, edge_index, W):
    return _kernel_impl(x, edge_index, W, N_NODES)
